# revision 41
# baseline (speedup 1.0000x reference)
"""Trainium2 Bass kernel for nn_BgeAttention (dense transformer block).

Sharding (8 NeuronCores): 2 batch groups x 4-way head/tensor parallel.
  core c: g = c//4 (batch), li = c%4 -> heads [4*li, 4*li+4)
  - QKV projections + attention for its 4 heads over the full 2048-token seq
  - partial o-proj (its 256 ctx dims) -> bf16 ReduceScatter(add) over the
    4-core group, each core keeping tokens [512*li, 512*(li+1))
  - LN1 + FFN (bf16 weights) + LN2 on its 512-token slice

v2 design notes:
  - x ships host-side pre-transposed+bf16 (xgT [1024,2048]) so the kernel
    never runs a PE transpose for QKV; FFN's A^T goes through the DMA xbar
    (dma_start_transpose) instead of PE transpose + copy.
  - attention inner loop: the head-pair score matmuls (K=64) issue
    adjacently at row groups (0,0)/(64,0) so both run concurrently in the
    PE array; the loop is paced by the ACT-engine exp stream (hard floor).
  - o-proj partials are stored bf16 and ReduceScattered bf16 (CCE adds in
    bf16); rs_out load + LN1 moved to the FFN prologue so the attention
    window has no collective-dependent loads, and the last RS chunk hides
    behind FFN weight DMA + LN1(qb0..2) + At transposes.
  - fc1 keeps all of h in SBUF; fc2 accumulates each output tile fully in
    PSUM over all 32 f-chunks (no per-fg DVE accumulate adds).
  - LN rstd = Rsqrt(var+eps) (one table set) + one Newton step on DVE for
    fp32-grade accuracy; act-table loads drop from 18 to ~4.
"""
import sys, os
sys.path.insert(0, '/opt/trn_rl_repo')
import numpy as np
import ml_dtypes
import concourse.bass as bass
import concourse.tile as tile
from concourse import bacc, mybir
from concourse.bass_utils import run_bass_kernel_spmd
from concourse.masks import make_identity

F32 = mybir.dt.float32
F32R = mybir.dt.float32r
BF16 = mybir.dt.bfloat16
AF = mybir.ActivationFunctionType
OP = mybir.AluOpType

S, D, HD, F = 2048, 1024, 64, 4096
GROUPS = [[0, 1, 2, 3], [4, 5, 6, 7]]
EPS = 1e-12

_CACHE = {}


def _bcast_ap(ap, p=128):
    return bass.AP(tensor=ap.tensor, offset=ap.offset, ap=[[0, p]] + list(ap.ap))


def _build(nrep=1):
    nc = bacc.Bacc("TRN2", target_bir_lowering=False, debug=False, num_devices=8)

    # weights arrive pre-transposed from _in_maps into partition-major
    # layouts so every DMA is one contiguous block per partition
    xgt = nc.dram_tensor("xgt", [D, S], BF16, kind="ExternalInput").ap()
    wq = nc.dram_tensor("wq", [128, 2048], BF16, kind="ExternalInput").ap()
    wk = nc.dram_tensor("wk", [128, 2048], BF16, kind="ExternalInput").ap()
    wv = nc.dram_tensor("wv", [128, 2048], BF16, kind="ExternalInput").ap()
    wo = nc.dram_tensor("wo", [256, D], BF16, kind="ExternalInput").ap()
    w1 = nc.dram_tensor("w1", [128, 4, 8192], BF16, kind="ExternalInput").ap()
    w2 = nc.dram_tensor("w2", [128, 4, 8192], BF16, kind="ExternalInput").ap()
    bq = nc.dram_tensor("bq", [128, 2], F32, kind="ExternalInput").ap()
    bk = nc.dram_tensor("bk", [128, 2], F32, kind="ExternalInput").ap()
    bv = nc.dram_tensor("bv", [256], F32, kind="ExternalInput").ap()
    bo = nc.dram_tensor("bo", [D], F32, kind="ExternalInput").ap()
    b1 = nc.dram_tensor("b1", [128, 32], F32, kind="ExternalInput").ap()
    b2 = nc.dram_tensor("b2", [D], F32, kind="ExternalInput").ap()
    ln1g = nc.dram_tensor("ln1g", [D], F32, kind="ExternalInput").ap()
    ln1b = nc.dram_tensor("ln1b", [D], F32, kind="ExternalInput").ap()
    ln2g = nc.dram_tensor("ln2g", [D], F32, kind="ExternalInput").ap()
    ln2b = nc.dram_tensor("ln2b", [D], F32, kind="ExternalInput").ap()
    out = nc.dram_tensor("out", [512, D], F32, kind="ExternalOutput").ap()

    RSDT = F32 if os.environ.get("BGE_RS_F32") else BF16
    rs_in = nc.dram_tensor("rs_in", [S, D], RSDT)
    rs_out = nc.dram_tensor("rs_out", [512, D], RSDT)

    t = locals()
    with tile.TileContext(nc) as tc:
        for _r in range(nrep):
            _emit(nc, tc, t)
    nc.compile()
    return nc


def _emit(nc, tc, t):
    from contextlib import ExitStack
    from itertools import cycle
    from collections import deque
    PH = os.environ.get("BGE_KERNEL_PHASES", "full")
    xgt, wq, wk, wv, wo, w1, w2 = t["xgt"], t["wq"], t["wk"], t["wv"], t["wo"], t["w1"], t["w2"]
    bq, bk, bv, bo, b1, b2 = t["bq"], t["bk"], t["bv"], t["bo"], t["b1"], t["b2"]
    ln1g, ln1b, ln2g, ln2b = t["ln1g"], t["ln1b"], t["ln2g"], t["ln2b"]
    out, rs_in, rs_out = t["out"], t["rs_in"], t["rs_out"]
    RSDT = t["RSDT"]
    do_rs = PH in ("paor", "full")

    with ExitStack() as top:
        const = top.enter_context(tc.tile_pool(name="const", bufs=1))
        stp = top.enter_context(tc.tile_pool(name="stp", bufs=2))

        ident = const.tile([128, 128], F32)
        make_identity(nc, ident[:])
        identb = const.tile([128, 128], BF16)
        nc.vector.tensor_copy(identb[:], ident[:])
        eps = const.tile([128, 1], F32)
        nc.vector.memset(eps[:], EPS)
        ones1f = const.tile([1, 64], F32)
        nc.vector.memset(ones1f[:], 1.0)
        ones1 = const.tile([1, 64], F32R)
        nc.vector.tensor_copy(ones1[:], ones1f[:])
        onesc = const.tile([128, 4, 1], F32)
        nc.vector.memset(onesc[:], 1.0)
        half_t = const.tile([128, 1], F32)
        nc.vector.memset(half_t[:], -0.5)

        def bc_tile(src, n, name, pool):
            # sync, not gpsimd: F-phase broadcasts emitted after the RS waits
            # in the Pool FIFO would gate the whole LN1 prologue on RS3
            tl = pool.tile([128, n], F32, name=name)
            nc.sync.dma_start(out=tl[:], in_=_bcast_ap(src))
            return tl

        bv_b = bc_tile(bv, 256, "bv_b", const)
        b1_sb = const.tile([128, 32], F32, name="b1_sb")
        nc.gpsimd.dma_start(out=b1_sb[:], in_=b1)
        bq_sb = const.tile([128, 2], F32, name="bq_sb")
        nc.gpsimd.dma_start(out=bq_sb[:], in_=bq)
        bk_sb = const.tile([128, 2], F32, name="bk_sb")
        nc.gpsimd.dma_start(out=bk_sb[:], in_=bk)

        def rstd_newton(dst, var_ap, n, pool):
            """dst[128,n] = rsqrt(var+eps): DVE reciprocal -> ACT Sqrt table
            (single 'sqrt' table set, no exp/ln thrash) -> one DVE Newton
            step y <- y*(1.5 - 0.5*(v+eps)*y^2) to recover fp32 accuracy
            (the sqrt table has a ~65536-ULP budget)."""
            ve = pool.tile([128, n], F32, name="ve")
            nc.vector.tensor_scalar_add(out=ve[:], in0=var_ap, scalar1=eps[:])
            rcp = pool.tile([128, n], F32, name="rcp")
            nc.vector.reciprocal(rcp[:], ve[:])
            y0 = pool.tile([128, n], F32, name="y0")
            nc.scalar.activation(out=y0[:], in_=rcp[:], func=AF.Sqrt)
            t1 = pool.tile([128, n], F32, name="t1")
            nc.vector.tensor_scalar(out=t1[:], in0=ve[:], scalar1=half_t[:],
                                    scalar2=0.0, op0=OP.mult, op1=OP.add)
            nc.vector.tensor_mul(out=t1[:], in0=t1[:], in1=y0[:])
            nc.vector.tensor_mul(out=t1[:], in0=t1[:], in1=y0[:])
            nc.vector.tensor_scalar_add(out=t1[:], in0=t1[:], scalar1=1.5)
            nc.vector.tensor_tensor(out=dst, in0=t1[:], in1=y0[:], op=OP.mult)

        with ExitStack() as pa:
            attp = pa.enter_context(tc.tile_pool(name="attp", bufs=1))
            Qt = [attp.tile([128, S], BF16, name=f"qt{i}") for i in range(2)]
            Kt = [attp.tile([128, S], BF16, name=f"kt{i}") for i in range(2)]
            Vaug = [attp.tile([128, 4, 65], BF16, name=f"va{kc}") for kc in range(16)]
            Ctx = [attp.tile([128, S], BF16, name=f"ctx{i}") for i in range(2)]
            wo_t = attp.tile([128, 2, D], BF16, name="wo_t")

            # ---------- Phase P: QKV projections (no transposes) ----------
            with ExitStack() as ph:
                xtp = ph.enter_context(tc.tile_pool(name="xtp", bufs=1))
                wp = ph.enter_context(tc.tile_pool(name="wp", bufs=1))
                psP = ph.enter_context(tc.tile_pool(name="psP", bufs=4, space="PSUM"))

                Xt = xtp.tile([128, 8, S], BF16, name="xt")
                wq_t = wp.tile([128, 8, 256], BF16, name="wq_t")
                wk_t = wp.tile([128, 8, 256], BF16, name="wk_t")
                wv_t = wp.tile([128, 8, 256], BF16, name="wv_t")
                # weights first (small), then x^T token-halves hh=0 for all
                # d-chunks (unlocks ts=0/1 QK + V kc 0..7), then hh=1, so PE
                # ramps after ~3.5MB instead of the full 5.5MB
                _e2 = cycle((nc.sync, nc.scalar, nc.gpsimd))
                for _wt, _w in ((wk_t, wk), (wq_t, wq), (wv_t, wv)):
                    for g4 in range(2):
                        next(_e2).dma_start(
                            out=_wt[:, g4 * 4:(g4 + 1) * 4, :],
                            in_=_w[:, g4 * 1024:(g4 + 1) * 1024])
                for hh in range(2):
                    for dc in range(8):
                        next(_e2).dma_start(
                            out=Xt[:, dc, hh * 1024:(hh + 1) * 1024],
                            in_=xgt[dc * 128:(dc + 1) * 128,
                                    hh * 1024:(hh + 1) * 1024])
                for dc2 in range(2):
                    next(_e2).dma_start(
                        out=wo_t[:, dc2, :], in_=wo[dc2 * 128:(dc2 + 1) * 128, :])

                # K and V first (attention qb0 needs ALL keys/values but only
                # Qt[:, 0:512]); Q last so qb0's scores can start while Q of
                # later query blocks still projects
                def proj_qk(w_t, b_sb, Dst, ts):
                    for oc in range(2):
                        pk = psP.tile([128, 512], F32, name="ps")
                        for dc in range(8):
                            nc.tensor.matmul(pk[:], w_t[:, dc, oc * 128:(oc + 1) * 128],
                                             Xt[:, dc, ts * 512:(ts + 1) * 512],
                                             start=(dc == 0), stop=(dc == 7))
                        nc.vector.tensor_scalar_add(
                            out=Dst[oc][:, ts * 512:(ts + 1) * 512], in0=pk[:],
                            scalar1=b_sb[:, oc:oc + 1])

                for ts in range(4):
                    proj_qk(wk_t, bk_sb, Kt, ts)
                    for tc4 in range(2):
                        kc = ts * 4 + tc4
                        pv = psP.tile([128, 256], F32, name="psv")
                        for dc in range(8):
                            nc.tensor.matmul(pv[:], Xt[:, dc, kc * 128:(kc + 1) * 128],
                                             wv_t[:, dc, :], start=(dc == 0), stop=(dc == 7))
                        nc.vector.tensor_tensor(
                            out=Vaug[kc][:, :, 0:64],
                            in0=pv[:].rearrange("p (h d) -> p h d", h=4),
                            in1=bv_b[:].rearrange("p (h d) -> p h d", h=4),
                            op=OP.add)
                        nc.vector.tensor_copy(Vaug[kc][:, :, 64:65], onesc[:])
                for ts in range(4):
                    for tc4 in range(2, 4):
                        kc = ts * 4 + tc4
                        pv = psP.tile([128, 256], F32, name="psv")
                        for dc in range(8):
                            nc.tensor.matmul(pv[:], Xt[:, dc, kc * 128:(kc + 1) * 128],
                                             wv_t[:, dc, :], start=(dc == 0), stop=(dc == 7))
                        nc.vector.tensor_tensor(
                            out=Vaug[kc][:, :, 0:64],
                            in0=pv[:].rearrange("p (h d) -> p h d", h=4),
                            in1=bv_b[:].rearrange("p (h d) -> p h d", h=4),
                            op=OP.add)
                        nc.vector.tensor_copy(Vaug[kc][:, :, 64:65], onesc[:])
                for ts in range(4):
                    proj_qk(wq_t, bq_sb, Qt, ts)

            # ---- Phase A: attention; o-proj/store/RS deferred into next qb ----
            if PH in ("pa", "pao", "paor", "paof", "full"):
                with ExitStack() as ph:
                    expp = ph.enter_context(tc.tile_pool(name="expp", bufs=3))
                    rzp = ph.enter_context(tc.tile_pool(name="rzp", bufs=2))
                    stgp = ph.enter_context(tc.tile_pool(name="stgp", bufs=2))
                    scP = ph.enter_context(tc.tile_pool(name="scP", bufs=2, space="PSUM"))
                    psO = ph.enter_context(tc.tile_pool(name="psO", bufs=1, space="PSUM"))
                    psB = ph.enter_context(tc.tile_pool(name="psB", bufs=2, space="PSUM"))
                    psC = ph.enter_context(tc.tile_pool(name="psC", bufs=1, space="PSUM"))
                    do_o = PH in ("pao", "paor", "paof", "full")
                    pending = deque()

                    def queue_oproj(qb):
                        """Defer qb's o-proj/store/RS-trigger as work items
                        drained two-at-a-time inside the NEXT qb's loop."""
                        if not do_o:
                            return
                        sA = stgp.tile([128, 4, D], RSDT, name="sA")

                        def mk_mm(q4, oh):
                            def go():
                                po = psO.tile([128, 512], F32, name="po")
                                tc16 = qb * 4 + q4
                                for dc2 in range(2):
                                    nc.tensor.matmul(
                                        po[:], Ctx[dc2][:, tc16 * 128:(tc16 + 1) * 128],
                                        wo_t[:, dc2, oh * 512:(oh + 1) * 512],
                                        start=(dc2 == 0), stop=(dc2 == 1))
                                nc.vector.tensor_copy(
                                    sA[:, q4, oh * 512:(oh + 1) * 512], po[:])
                            return go

                        def mk_store(q4):
                            def go():
                                nc.sync.dma_start(
                                    out=rs_in[(qb * 4 + q4) * 128:(qb * 4 + q4 + 1) * 128, :],
                                    in_=sA[:, q4, :])
                            return go

                        for q4 in range(4):
                            for oh in range(2):
                                pending.append(mk_mm(q4, oh))
                            pending.append(mk_store(q4))

                        def trig():
                            if do_rs:
                                nc.gpsimd.collective_compute(
                                    "ReduceScatter", OP.add,
                                    ins=[rs_in[qb * 512:(qb + 1) * 512, :]],
                                    outs=[rs_out[qb * 128:(qb + 1) * 128, :]],
                                    replica_groups=GROUPS)
                        pending.append(trig)

                    def drain(n):
                        for _ in range(n):
                            if pending:
                                pending.popleft()()

                    def norm_closure(qb, hp, avs):
                        """Softmax normalize (recip/bcast/mul), delayed past
                        the NEXT block's first score pair — emitted inline at
                        the boundary it head-of-line blocks the PE queue on
                        the DVE reciprocal latency. PE-matmul broadcast: the
                        Pool queue must stay free of norm work — it carries
                        the blocking ReduceScatter waits."""
                        def go():
                            for i in range(2):
                                rz = rzp.tile([1, 512], F32R, name="rz")
                                with nc.allow_low_precision(reason="f32r is full width"):
                                    nc.vector.reciprocal(rz[:], avs[i][64:65, :])
                                bcp = psC.tile([64, 512], F32, name="bcp")
                                nc.tensor.matmul(bcp[:], ones1[:], rz[:],
                                                 start=True, stop=True)
                                rzs = rzp.tile([64, 512], F32, name="rzs")
                                nc.vector.tensor_copy(rzs[:], bcp[:])
                                nc.vector.tensor_mul(
                                    out=Ctx[hp][i * 64:(i + 1) * 64,
                                                qb * 512:(qb + 1) * 512],
                                    in0=avs[i][0:64, :], in1=rzs[:])
                        return go

                    prev_norm = None
                    for qb in range(4):
                        for hp in range(2):
                            avs = [psB.tile([65, 512], F32, name="av") for i in range(2)]
                            es_prev = None
                            for kp in range(9):
                                if kp < 8:
                                    scs = [scP.tile([128, 1024], F32, name="sc2") for i in range(2)]
                                    # head-pair scores adjacent: K=64 row groups
                                    # (0,0)/(64,0) run concurrently in the array
                                    for half in range(2):
                                        kc = 2 * kp + half
                                        for i in range(2):
                                            nc.tensor.matmul(
                                                scs[i][:, half * 512:(half + 1) * 512],
                                                Kt[hp][i * 64:(i + 1) * 64, kc * 128:(kc + 1) * 128],
                                                Qt[hp][i * 64:(i + 1) * 64, qb * 512:(qb + 1) * 512],
                                                start=True, stop=True)
                                    es = []
                                    for i in range(2):
                                        e = expp.tile([128, 1024], BF16, name=f"e{i}")
                                        nc.scalar.activation(e[:], scs[i][:], AF.Exp)
                                        es.append(e)
                                if kp == 0 and prev_norm is not None:
                                    prev_norm()
                                    prev_norm = None
                                # AV delayed one step: the exp stream never
                                # waits for scores stuck behind AV in the FIFO
                                if kp >= 1:
                                    for i in range(2):
                                        for half in range(2):
                                            kc = 2 * (kp - 1) + half
                                            nc.tensor.matmul(
                                                avs[i][:], Vaug[kc][:, 2 * hp + i, :],
                                                es_prev[i][:, half * 512:(half + 1) * 512],
                                                start=(kc == 0), stop=(kc == 15))
                                    drain(3)
                                if kp < 8:
                                    es_prev = es
                            prev_norm = norm_closure(qb, hp, avs)
                        queue_oproj(qb)
                    if prev_norm is not None:
                        prev_norm()
                    while pending:
                        pending.popleft()()

        # =============== Phase F: LN1 + FFN + LN2 ===============
        if PH not in ("full", "paof"):
            return
        with ExitStack() as ph:
            accp = ph.enter_context(tc.tile_pool(name="accp", bufs=1))
            sbA = ph.enter_context(tc.tile_pool(name="sbA", bufs=1))
            w1p = ph.enter_context(tc.tile_pool(name="w1p", bufs=2))
            w2p = ph.enter_context(tc.tile_pool(name="w2p", bufs=1))
            hp_ = ph.enter_context(tc.tile_pool(name="hp", bufs=1))
            fmisc = ph.enter_context(tc.tile_pool(name="fmisc", bufs=2))
            psF = ph.enter_context(tc.tile_pool(name="psF", bufs=3, space="PSUM"))
            psD = ph.enter_context(tc.tile_pool(name="psD", bufs=3, space="PSUM"))

            At = [sbA.tile([128, 512], BF16, name=f"at{dc}") for dc in range(8)]
            ffn_acc = [accp.tile([128, D], F32, name=f"fa{i}") for i in range(4)]
            w2t = w2p.tile([128, 32, 1024], BF16, name="w2t")
            hts = hp_.tile([128, 32, 512], BF16, name="hts")

            # FFN weight stream starts immediately (covers the RS tail)
            _f2 = cycle((nc.scalar, nc.sync))

            def load_w1(fg):
                wt = w1p.tile([128, 8, 1024], BF16, name="w1t")
                for g4 in range(4):
                    next(_f2).dma_start(out=wt[:, g4 * 2:(g4 + 1) * 2, :],
                                        in_=w1[:, fg:fg + 1, g4 * 2048:(g4 + 1) * 2048])
                return wt

            w1ts = {0: load_w1(0)}

            # ---- prologue: per-qb LN1 as its RS lands; At via DMA xbar ----
            with ExitStack() as pg:
                plnc = pg.enter_context(tc.tile_pool(name="plnc", bufs=1))
                bo_b = bc_tile(bo, D, "bo_b", plnc)
                ln1g_b = bc_tile(ln1g, D, "ln1g_b", plnc)
                ln1b_b = bc_tile(ln1b, D, "ln1b_b", plnc)
                b2_b = bc_tile(b2, D, "b2_b", plnc)
                ln1b2_b = plnc.tile([128, D], F32, name="ln1b2_b")
                nc.vector.tensor_add(out=ln1b2_b[:], in0=ln1b_b[:], in1=b2_b[:])
                rawp = pg.enter_context(tc.tile_pool(name="rawp", bufs=2))
                prep = pg.enter_context(tc.tile_pool(name="prep", bufs=2))
                abfp = pg.enter_context(tc.tile_pool(name="abfp", bufs=1))
                psT = pg.enter_context(tc.tile_pool(name="psT", bufs=2, space="PSUM"))
                A_bf = [abfp.tile([128, D], BF16, name=f"ab{i}") for i in range(4)]

                for qb in range(4):
                    # both halves on SP: an RS-gated load on the ACT queue
                    # would head-of-line block attention exps behind the
                    # collective (Tile hoists it into the attention window)
                    raw = rawp.tile([128, D], RSDT, name="raw")
                    for hh in range(2):
                        nc.sync.dma_start(
                            out=raw[:, hh * 512:(hh + 1) * 512],
                            in_=rs_out[qb * 128:(qb + 1) * 128,
                                       hh * 512:(hh + 1) * 512])
                    rawf = prep.tile([128, D], F32, name="scr")
                    nc.vector.tensor_tensor(out=rawf[:], in0=raw[:], in1=bo_b[:],
                                            op=OP.add)
                    stats = stp.tile([128, 2, 6], F32, name="stats")
                    for sgi in range(2):
                        nc.vector.bn_stats(out=stats[:, sgi, :],
                                           in_=rawf[:, sgi * 512:(sgi + 1) * 512])
                    mv = stp.tile([128, 2], F32, name="mv")
                    nc.vector.bn_aggr(out=mv[:], in_=stats[:])
                    rstd = stp.tile([128, 1], F32, name="rstd")
                    rstd_newton(rstd[:], mv[:, 1:2], 1, stp)
                    pre = prep.tile([128, D], F32, name="scr")
                    nc.vector.tensor_scalar(out=pre[:], in0=rawf[:],
                                            scalar1=mv[:, 0:1], scalar2=rstd[:],
                                            op0=OP.subtract, op1=OP.mult)
                    nc.vector.tensor_mul(out=pre[:], in0=pre[:], in1=ln1g_b[:])
                    nc.vector.tensor_tensor(out=A_bf[qb][:], in0=pre[:],
                                            in1=ln1b_b[:], op=OP.add)
                    nc.vector.tensor_tensor(out=ffn_acc[qb][:], in0=pre[:],
                                            in1=ln1b2_b[:], op=OP.add)
                    # At via PE transpose (PE is idle here); copies on ACT
                    for dc in range(8):
                        pt = psT.tile([128, 128], BF16, name="pt")
                        nc.tensor.transpose(pt[:], A_bf[qb][:, dc * 128:(dc + 1) * 128],
                                            identb[:])
                        nc.scalar.copy(At[dc][:, qb * 128:(qb + 1) * 128], pt[:])

            # LN2/out staging opens after the prologue pools close so it
            # reuses their SBUF space
            lnc = ph.enter_context(tc.tile_pool(name="lnc", bufs=1))
            ln2g_b = bc_tile(ln2g, D, "ln2g_b", lnc)
            ln2b_b = bc_tile(ln2b, D, "ln2b_b", lnc)

            # ---- fc1: h = gelu(relu(At @ W1 + b1)), all of h kept in SBUF ----
            for fg in range(4):
                if fg < 3:
                    w1ts[fg + 1] = load_w1(fg + 1)
                for g4 in range(4):
                    next(_f2).dma_start(
                        out=w2t[:, fg * 8 + g4 * 2:fg * 8 + (g4 + 1) * 2, :],
                        in_=w2[:, fg:fg + 1, g4 * 2048:(g4 + 1) * 2048])
                w1c = w1ts.pop(fg)
                for fc8 in range(8):
                    fci = fg * 8 + fc8
                    # fg0/1 split at token 384: the qb0-2 part only needs
                    # RS0-2, so it runs during the RS3 tail window
                    segs = ((0, 384), (384, 512)) if fg < 2 else ((0, 512),)
                    for c0, c1 in segs:
                        phm = psF.tile([128, c1 - c0], F32, name="ps")
                        for dc in range(8):
                            nc.tensor.matmul(phm[:], w1c[:, dc, fc8 * 128:(fc8 + 1) * 128],
                                             At[dc][:, c0:c1], start=(dc == 0), stop=(dc == 7))
                        tmp = fmisc.tile([128, c1 - c0], BF16, name="tmp")
                        nc.vector.tensor_scalar(out=tmp[:], in0=phm[:],
                                                scalar1=b1_sb[:, fci:fci + 1], scalar2=0.0,
                                                op0=OP.add, op1=OP.max)
                        nc.scalar.activation(hts[:, fci, c0:c1], tmp[:], AF.Gelu)

            # ---- fc2: out tiles accumulate over all 32 f-chunks in PSUM ----
            oall = lnc.tile([128, 4, D], F32, name="oall")
            mv4 = lnc.tile([128, 4, 2], F32, name="mv4")
            for tc4 in range(4):
                for oh in range(2):
                    pacc = psD.tile([128, 512], F32, name="pac")
                    for fci in range(32):
                        nc.tensor.matmul(pacc[:],
                                         hts[:, fci, tc4 * 128:(tc4 + 1) * 128],
                                         w2t[:, fci, oh * 512:(oh + 1) * 512],
                                         start=(fci == 0), stop=(fci == 31))
                    dst = ffn_acc[tc4][:, oh * 512:(oh + 1) * 512]
                    nc.vector.tensor_add(out=dst, in0=dst, in1=pacc[:])
                # LN2 + store per tc4 immediately: overlaps the next tc4's
                # fc2 matmul stream instead of a serial batched tail
                stats = stp.tile([128, 2, 6], F32, name="st2")
                for sgi in range(2):
                    nc.vector.bn_stats(out=stats[:, sgi, :],
                                       in_=ffn_acc[tc4][:, sgi * 512:(sgi + 1) * 512])
                nc.vector.bn_aggr(out=mv4[:, tc4, :], in_=stats[:])
                rstd1 = lnc.tile([128, 4], F32, name=f"rstd{tc4}")
                rstd_newton(rstd1[:, 0:1], mv4[:, tc4, 1:2], 1, stp)
                acc = ffn_acc[tc4]
                dst4 = oall[:, tc4, :]
                for eng, c0, c1 in ((nc.vector, 0, 640), (nc.gpsimd, 640, 1024)):
                    eng.tensor_scalar(out=dst4[:, c0:c1], in0=acc[:, c0:c1],
                                      scalar1=mv4[:, tc4, 0:1],
                                      scalar2=rstd1[:, 0:1],
                                      op0=OP.subtract, op1=OP.mult)
                    eng.tensor_tensor(out=dst4[:, c0:c1], in0=dst4[:, c0:c1],
                                      in1=ln2g_b[:, c0:c1], op=OP.mult)
                    eng.tensor_tensor(out=dst4[:, c0:c1], in0=dst4[:, c0:c1],
                                      in1=ln2b_b[:, c0:c1], op=OP.add)
                for s4 in range(4):
                    (nc.sync, nc.gpsimd, nc.scalar, nc.sync)[s4].dma_start(
                        out=out[tc4 * 128:(tc4 + 1) * 128,
                                s4 * 256:(s4 + 1) * 256],
                        in_=oall[:, tc4, s4 * 256:(s4 + 1) * 256])


def _get_nc():
    if "nc" not in _CACHE:
        _CACHE["nc"] = _build()
    return _CACHE["nc"]


def _qkv_pmajor(w):
    # [1024, 256] -> [128, 2048]: w_t[p, g4*1024 + a*256 + f] = w[g4*512+a*128+p, f]
    return np.ascontiguousarray(
        w.reshape(2, 4, 128, 256).transpose(2, 0, 1, 3).reshape(128, 2048))


def _w1_pmajor(w1):
    # [1024, 4096] -> [128, 4, 8192]:
    # w1n[p, fg, g4*2048 + a*1024 + f] = w1[g4*256 + a*128 + p, fg*1024 + f]
    return np.ascontiguousarray(
        w1.reshape(4, 2, 128, 4, 1024).transpose(2, 3, 0, 1, 4).reshape(128, 4, 8192))


def _w2_pmajor(w2):
    # [4096, 1024] -> [128, 4, 8192]:
    # w2n[p, fg, g4*2048 + a*1024 + f] = w2[fg*1024 + g4*256 + a*128 + p, f]
    return np.ascontiguousarray(
        w2.reshape(4, 4, 2, 128, 1024).transpose(3, 0, 1, 2, 4).reshape(128, 4, 8192))


def _in_maps(inputs):
    x = np.asarray(inputs["x"], dtype=np.float32)
    w1n = _w1_pmajor(np.asarray(inputs["W1"], np.float32)).astype(ml_dtypes.bfloat16)
    w2n = _w2_pmajor(np.asarray(inputs["W2"], np.float32)).astype(ml_dtypes.bfloat16)
    maps = []
    for c in range(8):
        g, li = c // 4, c % 4
        cs = slice(256 * li, 256 * (li + 1))
        m = {
            "xgt": np.ascontiguousarray(x[g].T).astype(ml_dtypes.bfloat16),
            "wq": _qkv_pmajor(np.asarray(inputs["Wq"], np.float32)[:, cs] / 8.0).astype(ml_dtypes.bfloat16),
            "wk": _qkv_pmajor(np.asarray(inputs["Wk"], np.float32)[:, cs]).astype(ml_dtypes.bfloat16),
            "wv": _qkv_pmajor(np.asarray(inputs["Wv"], np.float32)[:, cs]).astype(ml_dtypes.bfloat16),
            "wo": np.ascontiguousarray(np.asarray(inputs["Wo"], np.float32)[cs, :]).astype(ml_dtypes.bfloat16),
            "w1": w1n,
            "w2": w2n,
            "bq": np.ascontiguousarray(
                np.asarray(inputs["bq"], np.float32)[cs].reshape(2, 128).T) / 8.0,
            "bk": np.ascontiguousarray(
                np.asarray(inputs["bk"], np.float32)[cs].reshape(2, 128).T),
            "bv": np.ascontiguousarray(np.asarray(inputs["bv"], np.float32)[cs]),
            "bo": np.asarray(inputs["bo"], np.float32),
            "b1": np.ascontiguousarray(
                np.asarray(inputs["b1"], np.float32).reshape(32, 128).T),
            "b2": np.asarray(inputs["b2"], np.float32),
            "ln1g": np.asarray(inputs["ln1_g"], np.float32),
            "ln1b": np.asarray(inputs["ln1_b"], np.float32),
            "ln2g": np.asarray(inputs["ln2_g"], np.float32),
            "ln2b": np.asarray(inputs["ln2_b"], np.float32),
        }
        maps.append(m)
    return maps


def run(inputs, trace=False):
    nc = _get_nc()
    res = run_bass_kernel_spmd(nc, _in_maps(inputs), list(range(8)), trace=trace)
    B = 2
    full = np.empty((B, S, D), np.float32)
    for c in range(8):
        g, li = c // 4, c % 4
        o = res.results[c]["out"]
        for j in range(4):
            full[g, j * 512 + li * 128: j * 512 + (li + 1) * 128, :] = \
                o[j * 128:(j + 1) * 128]
    return full, res


def kernel(**inputs):
    return run(inputs)[0]


# revision 43
# speedup vs baseline: 1.0956x; 1.0956x over previous
"""Trainium2 Bass kernel for nn_BgeAttention (dense transformer block).

Sharding (8 NeuronCores): 2 batch groups x 4-way head/tensor parallel.
  core c: g = c//4 (batch), li = c%4 -> heads [4*li, 4*li+4)
  - QKV projections + attention for its 4 heads over the full 2048-token seq
  - partial o-proj (its 256 ctx dims) -> bf16 ReduceScatter(add) over the
    4-core group, each core keeping tokens [512*li, 512*(li+1))
  - LN1 + FFN (bf16 weights) + LN2 on its 512-token slice

v2 design notes:
  - x ships host-side pre-transposed+bf16 (xgT [1024,2048]) so the kernel
    never runs a PE transpose for QKV; FFN's A^T goes through the DMA xbar
    (dma_start_transpose) instead of PE transpose + copy.
  - attention inner loop: the head-pair score matmuls (K=64) issue
    adjacently at row groups (0,0)/(64,0) so both run concurrently in the
    PE array; the loop is paced by the ACT-engine exp stream (hard floor).
  - o-proj partials are stored bf16 and ReduceScattered bf16 (CCE adds in
    bf16); rs_out load + LN1 moved to the FFN prologue so the attention
    window has no collective-dependent loads, and the last RS chunk hides
    behind FFN weight DMA + LN1(qb0..2) + At transposes.
  - fc1 keeps all of h in SBUF; fc2 accumulates each output tile fully in
    PSUM over all 32 f-chunks (no per-fg DVE accumulate adds).
  - LN rstd = Rsqrt(var+eps) (one table set) + one Newton step on DVE for
    fp32-grade accuracy; act-table loads drop from 18 to ~4.
"""
import sys, os
sys.path.insert(0, '/opt/trn_rl_repo')
import numpy as np
import ml_dtypes
import concourse.bass as bass
import concourse.tile as tile
from concourse import bacc, mybir
from concourse.bass_utils import run_bass_kernel_spmd
from concourse.masks import make_identity

F32 = mybir.dt.float32
F32R = mybir.dt.float32r
BF16 = mybir.dt.bfloat16
AF = mybir.ActivationFunctionType
OP = mybir.AluOpType

S, D, HD, F = 2048, 1024, 64, 4096
GROUPS = [[0, 1, 2, 3], [4, 5, 6, 7]]
EPS = 1e-12

_CACHE = {}


def _bcast_ap(ap, p=128):
    return bass.AP(tensor=ap.tensor, offset=ap.offset, ap=[[0, p]] + list(ap.ap))


def _build(nrep=1):
    nc = bacc.Bacc("TRN2", target_bir_lowering=False, debug=False, num_devices=8)

    # weights arrive pre-transposed from _in_maps into partition-major
    # layouts so every DMA is one contiguous block per partition
    xgt = nc.dram_tensor("xgt", [D, S], BF16, kind="ExternalInput").ap()
    wq = nc.dram_tensor("wq", [128, 2048], BF16, kind="ExternalInput").ap()
    wk = nc.dram_tensor("wk", [128, 2048], BF16, kind="ExternalInput").ap()
    wv = nc.dram_tensor("wv", [128, 2048], BF16, kind="ExternalInput").ap()
    wo = nc.dram_tensor("wo", [256, D], BF16, kind="ExternalInput").ap()
    w1 = nc.dram_tensor("w1", [128, 4, 8192], BF16, kind="ExternalInput").ap()
    w2 = nc.dram_tensor("w2", [128, 4, 8192], BF16, kind="ExternalInput").ap()
    bq = nc.dram_tensor("bq", [128, 2], F32, kind="ExternalInput").ap()
    bk = nc.dram_tensor("bk", [128, 2], F32, kind="ExternalInput").ap()
    bv = nc.dram_tensor("bv", [256], F32, kind="ExternalInput").ap()
    bo = nc.dram_tensor("bo", [D], F32, kind="ExternalInput").ap()
    b1 = nc.dram_tensor("b1", [128, 32], F32, kind="ExternalInput").ap()
    b2 = nc.dram_tensor("b2", [D], F32, kind="ExternalInput").ap()
    ln1g = nc.dram_tensor("ln1g", [D], F32, kind="ExternalInput").ap()
    ln1b = nc.dram_tensor("ln1b", [D], F32, kind="ExternalInput").ap()
    ln2g = nc.dram_tensor("ln2g", [D], F32, kind="ExternalInput").ap()
    ln2b = nc.dram_tensor("ln2b", [D], F32, kind="ExternalInput").ap()
    bob = nc.dram_tensor("bob", [128, D], F32, kind="ExternalInput").ap()
    l1gb = nc.dram_tensor("l1gb", [128, D], F32, kind="ExternalInput").ap()
    l1bb = nc.dram_tensor("l1bb", [128, D], F32, kind="ExternalInput").ap()
    l12b = nc.dram_tensor("l12b", [128, D], F32, kind="ExternalInput").ap()
    l2gb = nc.dram_tensor("l2gb", [128, D], F32, kind="ExternalInput").ap()
    l2bb = nc.dram_tensor("l2bb", [128, D], F32, kind="ExternalInput").ap()
    out = nc.dram_tensor("out", [512, D], F32, kind="ExternalOutput").ap()

    RSDT = F32 if os.environ.get("BGE_RS_F32") else BF16
    rs_in = nc.dram_tensor("rs_in", [S, D], RSDT)
    rs_out = nc.dram_tensor("rs_out", [512, D], RSDT)

    t = locals()
    with tile.TileContext(nc) as tc:
        for _r in range(nrep):
            _emit(nc, tc, t)
    nc.compile()
    return nc


def _emit(nc, tc, t):
    from contextlib import ExitStack
    from itertools import cycle
    from collections import deque
    PH = os.environ.get("BGE_KERNEL_PHASES", "full")
    xgt, wq, wk, wv, wo, w1, w2 = t["xgt"], t["wq"], t["wk"], t["wv"], t["wo"], t["w1"], t["w2"]
    bq, bk, bv, bo, b1, b2 = t["bq"], t["bk"], t["bv"], t["bo"], t["b1"], t["b2"]
    ln1g, ln1b, ln2g, ln2b = t["ln1g"], t["ln1b"], t["ln2g"], t["ln2b"]
    out, rs_in, rs_out = t["out"], t["rs_in"], t["rs_out"]
    bob, l1gb, l1bb, l12b = t["bob"], t["l1gb"], t["l1bb"], t["l12b"]
    l2gb, l2bb = t["l2gb"], t["l2bb"]
    RSDT = t["RSDT"]
    do_rs = PH in ("paor", "full")

    with ExitStack() as top:
        const = top.enter_context(tc.tile_pool(name="const", bufs=1))
        stp = top.enter_context(tc.tile_pool(name="stp", bufs=2))

        ident = const.tile([128, 128], F32)
        make_identity(nc, ident[:])
        identb = const.tile([128, 128], BF16)
        nc.vector.tensor_copy(identb[:], ident[:])
        eps = const.tile([128, 1], F32)
        nc.vector.memset(eps[:], EPS)
        ones1f = const.tile([1, 64], F32)
        nc.vector.memset(ones1f[:], 1.0)
        ones1 = const.tile([1, 64], F32R)
        nc.vector.tensor_copy(ones1[:], ones1f[:])
        onesc = const.tile([128, 4, 1], F32)
        nc.vector.memset(onesc[:], 1.0)
        half_t = const.tile([128, 1], F32)
        nc.vector.memset(half_t[:], -0.5)

        def bc_tile(src, n, name, pool):
            tl = pool.tile([128, n], F32, name=name)
            nc.gpsimd.dma_start(out=tl[:], in_=_bcast_ap(src))
            return tl

        bv_b = bc_tile(bv, 256, "bv_b", const)
        b1_sb = const.tile([128, 32], F32, name="b1_sb")
        nc.gpsimd.dma_start(out=b1_sb[:], in_=b1)
        bq_sb = const.tile([128, 2], F32, name="bq_sb")
        nc.gpsimd.dma_start(out=bq_sb[:], in_=bq)
        bk_sb = const.tile([128, 2], F32, name="bk_sb")
        nc.gpsimd.dma_start(out=bk_sb[:], in_=bk)

        def rstd_newton(dst, var_ap, n, pool):
            """dst[128,n] = rsqrt(var+eps): DVE reciprocal -> ACT Sqrt table
            (single 'sqrt' table set, no exp/ln thrash) -> one DVE Newton
            step y <- y*(1.5 - 0.5*(v+eps)*y^2) to recover fp32 accuracy
            (the sqrt table has a ~65536-ULP budget)."""
            ve = pool.tile([128, n], F32, name="ve")
            nc.vector.tensor_scalar_add(out=ve[:], in0=var_ap, scalar1=eps[:])
            rcp = pool.tile([128, n], F32, name="rcp")
            nc.vector.reciprocal(rcp[:], ve[:])
            y0 = pool.tile([128, n], F32, name="y0")
            nc.scalar.activation(out=y0[:], in_=rcp[:], func=AF.Sqrt)
            t1 = pool.tile([128, n], F32, name="t1")
            nc.vector.tensor_scalar(out=t1[:], in0=ve[:], scalar1=half_t[:],
                                    scalar2=0.0, op0=OP.mult, op1=OP.add)
            nc.vector.tensor_mul(out=t1[:], in0=t1[:], in1=y0[:])
            nc.vector.tensor_mul(out=t1[:], in0=t1[:], in1=y0[:])
            nc.vector.tensor_scalar_add(out=t1[:], in0=t1[:], scalar1=1.5)
            nc.vector.tensor_tensor(out=dst, in0=t1[:], in1=y0[:], op=OP.mult)

        with ExitStack() as pa:
            attp = pa.enter_context(tc.tile_pool(name="attp", bufs=1))
            Qt = [attp.tile([128, S], BF16, name=f"qt{i}") for i in range(2)]
            Kt = [attp.tile([128, S], BF16, name=f"kt{i}") for i in range(2)]
            Vaug = [attp.tile([128, 4, 65], BF16, name=f"va{kc}") for kc in range(16)]
            Ctx = [attp.tile([128, S], BF16, name=f"ctx{i}") for i in range(2)]
            wo_t = attp.tile([128, 2, D], BF16, name="wo_t")

            # ---------- Phase P: QKV projections (no transposes) ----------
            with ExitStack() as ph:
                xtp = ph.enter_context(tc.tile_pool(name="xtp", bufs=1))
                wp = ph.enter_context(tc.tile_pool(name="wp", bufs=1))
                psP = ph.enter_context(tc.tile_pool(name="psP", bufs=4, space="PSUM"))

                Xt = xtp.tile([128, 8, S], BF16, name="xt")
                wq_t = wp.tile([128, 8, 256], BF16, name="wq_t")
                wk_t = wp.tile([128, 8, 256], BF16, name="wk_t")
                wv_t = wp.tile([128, 8, 256], BF16, name="wv_t")
                # weights first (small), then x^T token-halves hh=0 for all
                # d-chunks (unlocks ts=0/1 QK + V kc 0..7), then hh=1, so PE
                # ramps after ~3.5MB instead of the full 5.5MB
                _e2 = cycle((nc.sync, nc.scalar, nc.gpsimd))
                for _wt, _w in ((wk_t, wk), (wq_t, wq), (wv_t, wv)):
                    for g4 in range(2):
                        next(_e2).dma_start(
                            out=_wt[:, g4 * 4:(g4 + 1) * 4, :],
                            in_=_w[:, g4 * 1024:(g4 + 1) * 1024])
                for hh in range(2):
                    for dc in range(8):
                        next(_e2).dma_start(
                            out=Xt[:, dc, hh * 1024:(hh + 1) * 1024],
                            in_=xgt[dc * 128:(dc + 1) * 128,
                                    hh * 1024:(hh + 1) * 1024])
                for dc2 in range(2):
                    next(_e2).dma_start(
                        out=wo_t[:, dc2, :], in_=wo[dc2 * 128:(dc2 + 1) * 128, :])

                # K and V first (attention qb0 needs ALL keys/values but only
                # Qt[:, 0:512]); Q last so qb0's scores can start while Q of
                # later query blocks still projects
                def proj_qk(w_t, b_sb, Dst, ts):
                    for oc in range(2):
                        pk = psP.tile([128, 512], F32, name="ps")
                        for dc in range(8):
                            nc.tensor.matmul(pk[:], w_t[:, dc, oc * 128:(oc + 1) * 128],
                                             Xt[:, dc, ts * 512:(ts + 1) * 512],
                                             start=(dc == 0), stop=(dc == 7))
                        nc.vector.tensor_scalar_add(
                            out=Dst[oc][:, ts * 512:(ts + 1) * 512], in0=pk[:],
                            scalar1=b_sb[:, oc:oc + 1])

                for ts in range(4):
                    proj_qk(wk_t, bk_sb, Kt, ts)
                    for tc4 in range(2):
                        kc = ts * 4 + tc4
                        pv = psP.tile([128, 256], F32, name="psv")
                        for dc in range(8):
                            nc.tensor.matmul(pv[:], Xt[:, dc, kc * 128:(kc + 1) * 128],
                                             wv_t[:, dc, :], start=(dc == 0), stop=(dc == 7))
                        nc.vector.tensor_tensor(
                            out=Vaug[kc][:, :, 0:64],
                            in0=pv[:].rearrange("p (h d) -> p h d", h=4),
                            in1=bv_b[:].rearrange("p (h d) -> p h d", h=4),
                            op=OP.add)
                        nc.vector.tensor_copy(Vaug[kc][:, :, 64:65], onesc[:])
                for ts in range(4):
                    for tc4 in range(2, 4):
                        kc = ts * 4 + tc4
                        pv = psP.tile([128, 256], F32, name="psv")
                        for dc in range(8):
                            nc.tensor.matmul(pv[:], Xt[:, dc, kc * 128:(kc + 1) * 128],
                                             wv_t[:, dc, :], start=(dc == 0), stop=(dc == 7))
                        nc.vector.tensor_tensor(
                            out=Vaug[kc][:, :, 0:64],
                            in0=pv[:].rearrange("p (h d) -> p h d", h=4),
                            in1=bv_b[:].rearrange("p (h d) -> p h d", h=4),
                            op=OP.add)
                        nc.vector.tensor_copy(Vaug[kc][:, :, 64:65], onesc[:])
                for ts in range(4):
                    proj_qk(wq_t, bq_sb, Qt, ts)

            # ---- Phase A: attention; o-proj/store/RS deferred into next qb ----
            if PH in ("pa", "pao", "paor", "paof", "full"):
                with ExitStack() as ph:
                    expp = ph.enter_context(tc.tile_pool(name="expp", bufs=3))
                    rzp = ph.enter_context(tc.tile_pool(name="rzp", bufs=2))
                    stgp = ph.enter_context(tc.tile_pool(name="stgp", bufs=2))
                    scP = ph.enter_context(tc.tile_pool(name="scP", bufs=2, space="PSUM"))
                    psO = ph.enter_context(tc.tile_pool(name="psO", bufs=1, space="PSUM"))
                    psB = ph.enter_context(tc.tile_pool(name="psB", bufs=2, space="PSUM"))
                    psC = ph.enter_context(tc.tile_pool(name="psC", bufs=1, space="PSUM"))
                    do_o = PH in ("pao", "paor", "paof", "full")
                    pending = deque()

                    def queue_oproj(qb):
                        """Defer qb's o-proj/store/RS-trigger as work items
                        drained two-at-a-time inside the NEXT qb's loop."""
                        if not do_o:
                            return
                        sA = stgp.tile([128, 4, D], RSDT, name="sA")

                        def mk_mm(q4, oh):
                            def go():
                                po = psO.tile([128, 512], F32, name="po")
                                tc16 = qb * 4 + q4
                                for dc2 in range(2):
                                    nc.tensor.matmul(
                                        po[:], Ctx[dc2][:, tc16 * 128:(tc16 + 1) * 128],
                                        wo_t[:, dc2, oh * 512:(oh + 1) * 512],
                                        start=(dc2 == 0), stop=(dc2 == 1))
                                nc.vector.tensor_copy(
                                    sA[:, q4, oh * 512:(oh + 1) * 512], po[:])
                            return go

                        def mk_store(q4):
                            def go():
                                nc.sync.dma_start(
                                    out=rs_in[(qb * 4 + q4) * 128:(qb * 4 + q4 + 1) * 128, :],
                                    in_=sA[:, q4, :])
                            return go

                        for q4 in range(4):
                            for oh in range(2):
                                pending.append(mk_mm(q4, oh))
                            pending.append(mk_store(q4))

                        def trig():
                            if do_rs:
                                nc.gpsimd.collective_compute(
                                    "ReduceScatter", OP.add,
                                    ins=[rs_in[qb * 512:(qb + 1) * 512, :]],
                                    outs=[rs_out[qb * 128:(qb + 1) * 128, :]],
                                    replica_groups=GROUPS)
                        pending.append(trig)

                    def drain(n):
                        for _ in range(n):
                            if pending:
                                pending.popleft()()

                    def norm_closure(qb, hp, avs):
                        """Softmax normalize (recip/bcast/mul), delayed past
                        the NEXT block's first score pair — emitted inline at
                        the boundary it head-of-line blocks the PE queue on
                        the DVE reciprocal latency. PE-matmul broadcast: the
                        Pool queue must stay free of norm work — it carries
                        the blocking ReduceScatter waits."""
                        def go():
                            for i in range(2):
                                rz = rzp.tile([1, 512], F32R, name="rz")
                                with nc.allow_low_precision(reason="f32r is full width"):
                                    nc.vector.reciprocal(rz[:], avs[i][64:65, :])
                                bcp = psC.tile([64, 512], F32, name="bcp")
                                nc.tensor.matmul(bcp[:], ones1[:], rz[:],
                                                 start=True, stop=True)
                                rzs = rzp.tile([64, 512], F32, name="rzs")
                                nc.vector.tensor_copy(rzs[:], bcp[:])
                                nc.vector.tensor_mul(
                                    out=Ctx[hp][i * 64:(i + 1) * 64,
                                                qb * 512:(qb + 1) * 512],
                                    in0=avs[i][0:64, :], in1=rzs[:])
                        return go

                    prev_norm = None
                    for qb in range(4):
                        for hp in range(2):
                            avs = [psB.tile([65, 512], F32, name="av") for i in range(2)]
                            es_prev = None
                            for kp in range(9):
                                if kp < 8:
                                    scs = [scP.tile([128, 1024], F32, name="sc2") for i in range(2)]
                                    # head-pair scores adjacent: K=64 row groups
                                    # (0,0)/(64,0) run concurrently in the array
                                    for half in range(2):
                                        kc = 2 * kp + half
                                        for i in range(2):
                                            nc.tensor.matmul(
                                                scs[i][:, half * 512:(half + 1) * 512],
                                                Kt[hp][i * 64:(i + 1) * 64, kc * 128:(kc + 1) * 128],
                                                Qt[hp][i * 64:(i + 1) * 64, qb * 512:(qb + 1) * 512],
                                                start=True, stop=True)
                                    es = []
                                    for i in range(2):
                                        e = expp.tile([128, 1024], BF16, name=f"e{i}")
                                        nc.scalar.activation(e[:], scs[i][:], AF.Exp)
                                        es.append(e)
                                if kp == 0 and prev_norm is not None:
                                    prev_norm()
                                    prev_norm = None
                                # AV delayed one step: the exp stream never
                                # waits for scores stuck behind AV in the FIFO
                                if kp >= 1:
                                    for i in range(2):
                                        for half in range(2):
                                            kc = 2 * (kp - 1) + half
                                            nc.tensor.matmul(
                                                avs[i][:], Vaug[kc][:, 2 * hp + i, :],
                                                es_prev[i][:, half * 512:(half + 1) * 512],
                                                start=(kc == 0), stop=(kc == 15))
                                    drain(3)
                                if kp < 8:
                                    es_prev = es
                            prev_norm = norm_closure(qb, hp, avs)
                        queue_oproj(qb)
                    if prev_norm is not None:
                        prev_norm()
                    while pending:
                        pending.popleft()()

        # =============== Phase F: LN1 + FFN + LN2 ===============
        if PH not in ("full", "paof"):
            return
        with ExitStack() as ph:
            accp = ph.enter_context(tc.tile_pool(name="accp", bufs=1))
            sbA = ph.enter_context(tc.tile_pool(name="sbA", bufs=1))
            w1p = ph.enter_context(tc.tile_pool(name="w1p", bufs=2))
            w2p = ph.enter_context(tc.tile_pool(name="w2p", bufs=1))
            hp_ = ph.enter_context(tc.tile_pool(name="hp", bufs=1))
            fmisc = ph.enter_context(tc.tile_pool(name="fmisc", bufs=2))
            psF = ph.enter_context(tc.tile_pool(name="psF", bufs=3, space="PSUM"))
            psD = ph.enter_context(tc.tile_pool(name="psD", bufs=3, space="PSUM"))

            At = [sbA.tile([128, 512], BF16, name=f"at{dc}") for dc in range(8)]
            ffn_acc = [accp.tile([128, D], F32, name=f"fa{i}") for i in range(4)]
            w2t = w2p.tile([128, 32, 1024], BF16, name="w2t")
            hts = hp_.tile([128, 32, 512], BF16, name="hts")

            # FFN weight stream starts immediately (covers the RS tail)
            _f2 = cycle((nc.scalar, nc.sync))

            def load_w1(fg):
                wt = w1p.tile([128, 8, 1024], BF16, name="w1t")
                for g4 in range(4):
                    next(_f2).dma_start(out=wt[:, g4 * 2:(g4 + 1) * 2, :],
                                        in_=w1[:, fg:fg + 1, g4 * 2048:(g4 + 1) * 2048])
                return wt

            w1ts = {0: load_w1(0)}

            # ---- prologue: per-qb LN1 as its RS lands; At via DMA xbar ----
            with ExitStack() as pg:
                plnc = pg.enter_context(tc.tile_pool(name="plnc", bufs=1))
                bo_b = plnc.tile([128, D], F32, name="bo_b")
                ln1g_b = plnc.tile([128, D], F32, name="ln1g_b")
                ln1b_b = plnc.tile([128, D], F32, name="ln1b_b")
                ln1b2_b = plnc.tile([128, D], F32, name="ln1b2_b")
                for _tl, _src in ((bo_b, bob), (ln1g_b, l1gb),
                                  (ln1b_b, l1bb), (ln1b2_b, l12b)):
                    next(_f2).dma_start(out=_tl[:], in_=_src)
                rawp = pg.enter_context(tc.tile_pool(name="rawp", bufs=2))
                prep = pg.enter_context(tc.tile_pool(name="prep", bufs=2))
                abfp = pg.enter_context(tc.tile_pool(name="abfp", bufs=1))
                psT = pg.enter_context(tc.tile_pool(name="psT", bufs=2, space="PSUM"))
                A_bf = [abfp.tile([128, D], BF16, name=f"ab{i}") for i in range(4)]

                for qb in range(4):
                    # both halves on SP: an RS-gated load on the ACT queue
                    # would head-of-line block attention exps behind the
                    # collective (Tile hoists it into the attention window)
                    raw = rawp.tile([128, D], RSDT, name="raw")
                    for hh in range(2):
                        nc.sync.dma_start(
                            out=raw[:, hh * 512:(hh + 1) * 512],
                            in_=rs_out[qb * 128:(qb + 1) * 128,
                                       hh * 512:(hh + 1) * 512])
                    rawf = prep.tile([128, D], F32, name="scr")
                    nc.vector.tensor_tensor(out=rawf[:], in0=raw[:], in1=bo_b[:],
                                            op=OP.add)
                    stats = stp.tile([128, 2, 6], F32, name="stats")
                    for sgi in range(2):
                        nc.vector.bn_stats(out=stats[:, sgi, :],
                                           in_=rawf[:, sgi * 512:(sgi + 1) * 512])
                    mv = stp.tile([128, 2], F32, name="mv")
                    nc.vector.bn_aggr(out=mv[:], in_=stats[:])
                    rstd = stp.tile([128, 1], F32, name="rstd")
                    rstd_newton(rstd[:], mv[:, 1:2], 1, stp)
                    pre = prep.tile([128, D], F32, name="scr")
                    nc.vector.tensor_scalar(out=pre[:], in0=rawf[:],
                                            scalar1=mv[:, 0:1], scalar2=rstd[:],
                                            op0=OP.subtract, op1=OP.mult)
                    nc.vector.tensor_mul(out=pre[:], in0=pre[:], in1=ln1g_b[:])
                    nc.vector.tensor_tensor(out=A_bf[qb][:], in0=pre[:],
                                            in1=ln1b_b[:], op=OP.add)
                    nc.vector.tensor_tensor(out=ffn_acc[qb][:], in0=pre[:],
                                            in1=ln1b2_b[:], op=OP.add)
                    # At via PE transpose (PE is idle here); copies on ACT
                    for dc in range(8):
                        pt = psT.tile([128, 128], BF16, name="pt")
                        nc.tensor.transpose(pt[:], A_bf[qb][:, dc * 128:(dc + 1) * 128],
                                            identb[:])
                        nc.scalar.copy(At[dc][:, qb * 128:(qb + 1) * 128], pt[:])

            # LN2/out staging opens after the prologue pools close so it
            # reuses their SBUF space
            lnc = ph.enter_context(tc.tile_pool(name="lnc", bufs=1))
            ln2g_b = lnc.tile([128, D], F32, name="ln2g_b")
            nc.scalar.dma_start(out=ln2g_b[:], in_=l2gb)
            ln2b_b = lnc.tile([128, D], F32, name="ln2b_b")
            nc.sync.dma_start(out=ln2b_b[:], in_=l2bb)

            # ---- fc1: h = gelu(relu(At @ W1 + b1)), all of h kept in SBUF ----
            for fg in range(4):
                if fg < 3:
                    w1ts[fg + 1] = load_w1(fg + 1)
                for g4 in range(4):
                    next(_f2).dma_start(
                        out=w2t[:, fg * 8 + g4 * 2:fg * 8 + (g4 + 1) * 2, :],
                        in_=w2[:, fg:fg + 1, g4 * 2048:(g4 + 1) * 2048])
                w1c = w1ts.pop(fg)
                for fc8 in range(8):
                    fci = fg * 8 + fc8
                    # fg0/1 split at token 384: the qb0-2 part only needs
                    # RS0-2, so it runs during the RS3 tail window
                    segs = ((0, 384), (384, 512)) if fg < 2 else ((0, 512),)
                    for c0, c1 in segs:
                        phm = psF.tile([128, c1 - c0], F32, name="ps")
                        for dc in range(8):
                            nc.tensor.matmul(phm[:], w1c[:, dc, fc8 * 128:(fc8 + 1) * 128],
                                             At[dc][:, c0:c1], start=(dc == 0), stop=(dc == 7))
                        tmp = fmisc.tile([128, c1 - c0], BF16, name="tmp")
                        nc.vector.tensor_scalar(out=tmp[:], in0=phm[:],
                                                scalar1=b1_sb[:, fci:fci + 1], scalar2=0.0,
                                                op0=OP.add, op1=OP.max)
                        nc.scalar.activation(hts[:, fci, c0:c1], tmp[:], AF.Gelu)

            # ---- fc2: out tiles accumulate over all 32 f-chunks in PSUM ----
            oall = lnc.tile([128, 4, D], F32, name="oall")
            mv4 = lnc.tile([128, 4, 2], F32, name="mv4")
            for tc4 in range(4):
                for oh in range(2):
                    pacc = psD.tile([128, 512], F32, name="pac")
                    for fci in range(32):
                        nc.tensor.matmul(pacc[:],
                                         hts[:, fci, tc4 * 128:(tc4 + 1) * 128],
                                         w2t[:, fci, oh * 512:(oh + 1) * 512],
                                         start=(fci == 0), stop=(fci == 31))
                    dst = ffn_acc[tc4][:, oh * 512:(oh + 1) * 512]
                    nc.vector.tensor_add(out=dst, in0=dst, in1=pacc[:])
                # LN2 + store per tc4 immediately: overlaps the next tc4's
                # fc2 matmul stream instead of a serial batched tail
                stats = stp.tile([128, 2, 6], F32, name="st2")
                for sgi in range(2):
                    nc.vector.bn_stats(out=stats[:, sgi, :],
                                       in_=ffn_acc[tc4][:, sgi * 512:(sgi + 1) * 512])
                nc.vector.bn_aggr(out=mv4[:, tc4, :], in_=stats[:])
                rstd1 = lnc.tile([128, 4], F32, name=f"rstd{tc4}")
                rstd_newton(rstd1[:, 0:1], mv4[:, tc4, 1:2], 1, stp)
                acc = ffn_acc[tc4]
                dst4 = oall[:, tc4, :]
                for eng, c0, c1 in ((nc.vector, 0, 640), (nc.gpsimd, 640, 1024)):
                    eng.tensor_scalar(out=dst4[:, c0:c1], in0=acc[:, c0:c1],
                                      scalar1=mv4[:, tc4, 0:1],
                                      scalar2=rstd1[:, 0:1],
                                      op0=OP.subtract, op1=OP.mult)
                    eng.tensor_tensor(out=dst4[:, c0:c1], in0=dst4[:, c0:c1],
                                      in1=ln2g_b[:, c0:c1], op=OP.mult)
                    eng.tensor_tensor(out=dst4[:, c0:c1], in0=dst4[:, c0:c1],
                                      in1=ln2b_b[:, c0:c1], op=OP.add)
                for s4 in range(4):
                    (nc.sync, nc.gpsimd, nc.scalar, nc.sync)[s4].dma_start(
                        out=out[tc4 * 128:(tc4 + 1) * 128,
                                s4 * 256:(s4 + 1) * 256],
                        in_=oall[:, tc4, s4 * 256:(s4 + 1) * 256])


def _get_nc():
    if "nc" not in _CACHE:
        _CACHE["nc"] = _build()
    return _CACHE["nc"]


def _qkv_pmajor(w):
    # [1024, 256] -> [128, 2048]: w_t[p, g4*1024 + a*256 + f] = w[g4*512+a*128+p, f]
    return np.ascontiguousarray(
        w.reshape(2, 4, 128, 256).transpose(2, 0, 1, 3).reshape(128, 2048))


def _w1_pmajor(w1):
    # [1024, 4096] -> [128, 4, 8192]:
    # w1n[p, fg, g4*2048 + a*1024 + f] = w1[g4*256 + a*128 + p, fg*1024 + f]
    return np.ascontiguousarray(
        w1.reshape(4, 2, 128, 4, 1024).transpose(2, 3, 0, 1, 4).reshape(128, 4, 8192))


def _w2_pmajor(w2):
    # [4096, 1024] -> [128, 4, 8192]:
    # w2n[p, fg, g4*2048 + a*1024 + f] = w2[fg*1024 + g4*256 + a*128 + p, f]
    return np.ascontiguousarray(
        w2.reshape(4, 4, 2, 128, 1024).transpose(3, 0, 1, 2, 4).reshape(128, 4, 8192))


def _in_maps(inputs):
    x = np.asarray(inputs["x"], dtype=np.float32)
    w1n = _w1_pmajor(np.asarray(inputs["W1"], np.float32)).astype(ml_dtypes.bfloat16)
    w2n = _w2_pmajor(np.asarray(inputs["W2"], np.float32)).astype(ml_dtypes.bfloat16)
    maps = []
    for c in range(8):
        g, li = c // 4, c % 4
        cs = slice(256 * li, 256 * (li + 1))
        m = {
            "xgt": np.ascontiguousarray(x[g].T).astype(ml_dtypes.bfloat16),
            "wq": _qkv_pmajor(np.asarray(inputs["Wq"], np.float32)[:, cs] / 8.0).astype(ml_dtypes.bfloat16),
            "wk": _qkv_pmajor(np.asarray(inputs["Wk"], np.float32)[:, cs]).astype(ml_dtypes.bfloat16),
            "wv": _qkv_pmajor(np.asarray(inputs["Wv"], np.float32)[:, cs]).astype(ml_dtypes.bfloat16),
            "wo": np.ascontiguousarray(np.asarray(inputs["Wo"], np.float32)[cs, :]).astype(ml_dtypes.bfloat16),
            "w1": w1n,
            "w2": w2n,
            "bq": np.ascontiguousarray(
                np.asarray(inputs["bq"], np.float32)[cs].reshape(2, 128).T) / 8.0,
            "bk": np.ascontiguousarray(
                np.asarray(inputs["bk"], np.float32)[cs].reshape(2, 128).T),
            "bv": np.ascontiguousarray(np.asarray(inputs["bv"], np.float32)[cs]),
            "bo": np.asarray(inputs["bo"], np.float32),
            "b1": np.ascontiguousarray(
                np.asarray(inputs["b1"], np.float32).reshape(32, 128).T),
            "b2": np.asarray(inputs["b2"], np.float32),
            "ln1g": np.asarray(inputs["ln1_g"], np.float32),
            "ln1b": np.asarray(inputs["ln1_b"], np.float32),
            "ln2g": np.asarray(inputs["ln2_g"], np.float32),
            "ln2b": np.asarray(inputs["ln2_b"], np.float32),
            "bob": np.tile(np.asarray(inputs["bo"], np.float32), (128, 1)),
            "l1gb": np.tile(np.asarray(inputs["ln1_g"], np.float32), (128, 1)),
            "l1bb": np.tile(np.asarray(inputs["ln1_b"], np.float32), (128, 1)),
            "l12b": np.tile(np.asarray(inputs["ln1_b"], np.float32)
                            + np.asarray(inputs["b2"], np.float32), (128, 1)),
            "l2gb": np.tile(np.asarray(inputs["ln2_g"], np.float32), (128, 1)),
            "l2bb": np.tile(np.asarray(inputs["ln2_b"], np.float32), (128, 1)),
        }
        maps.append(m)
    return maps


def run(inputs, trace=False):
    nc = _get_nc()
    res = run_bass_kernel_spmd(nc, _in_maps(inputs), list(range(8)), trace=trace)
    B = 2
    full = np.empty((B, S, D), np.float32)
    for c in range(8):
        g, li = c // 4, c % 4
        o = res.results[c]["out"]
        for j in range(4):
            full[g, j * 512 + li * 128: j * 512 + (li + 1) * 128, :] = \
                o[j * 128:(j + 1) * 128]
    return full, res


def kernel(**inputs):
    return run(inputs)[0]
